# revision 5
# baseline (speedup 1.0000x reference)
"""Trainium2 Bass kernel for the 1x1-conv attention block + groupnorm-swish.

Reference computation (B=2, C=128, spatial 16^3 -> N=4096):
    q = wq@query + bq; k = wk@key + bk; v = wv@value + bv   (per batch, [C, N])
    S[i, j] = sum_c q[c,i] k[c,j]; P = softmax_j(S)
    h[c, i] = sum_j v[c,j] P[i,j]
    x = wo@h + bo + value
    out = silu(group_norm(x) * gamma + beta)   (G=32 groups of 4 channels)

Sharding: 8 cores = 2 batches x 4 query-token chunks of 1024 (sequence
parallel). Each core computes the k/v projections for its full batch
(replicated within the batch's 4-core group), its own S^T/softmax/PV chunk,
and group-norm partial sums; one tiny AllReduce produces full-batch group
statistics.

Key layout choices:
- Scores are computed TRANSPOSED (S^T[j, i] = k_tile^T @ q) so the PV
  contraction over j needs no transposes of the softmax output.
- Softmax max-subtraction is skipped (logits ~N(0, 14); exp stays in fp32
  range) and the denominator is accumulated ON THE TENSOR ENGINE: an all-ones
  stationary matrix against each exp tile accumulates sum_j exp[j, i] into a
  dedicated PSUM tile, already broadcast across partitions.
- The denominator is applied AFTER the output projection (column scaling
  commutes with channel mixing); the v bias folds into bo_eff = wo@bv + bo on
  the host.
- v^T is produced directly as matmul(value_tile, wv^T) in bf16 (no separate
  v projection, no transposes).
"""

import sys
import types

import ml_dtypes
import numpy as np

# The axon NTFF-profile hook module is absent from this image's antenv
# package; concourse imports it unconditionally when tracing. Install a
# functional shim (used by the test harness; harmless otherwise).
try:
    import antenv.axon_hooks  # noqa: F401
except ImportError:
    import antenv

    _mod = types.ModuleType("antenv.axon_hooks")
    _hook_box = [None]
    _mod.set_axon_ntff_profile_hook = lambda h: _hook_box.__setitem__(0, h)
    _mod.get_axon_ntff_profile_hook = lambda: _hook_box[0]
    sys.modules["antenv.axon_hooks"] = _mod
    antenv.axon_hooks = _mod
    try:
        from trn_agent_boot.trn_boot import _ntff_profile_via_ctypes

        _mod.set_axon_ntff_profile_hook(
            _ntff_profile_via_ctypes("/opt/axon/libaxon_pjrt.so")
        )
    except Exception:
        pass

import concourse.tile as tile
from concourse import bacc, mybir
from concourse.bass_utils import run_bass_kernel_spmd

B = 2
C = 128
N = 4096
NCORES = 8
CHUNKS = 4  # query-token chunks per batch
NC = N // CHUNKS  # 1024 tokens per core
JT = N // 128  # 32 key tiles of 128
G = 32  # groupnorm groups
EPS = 1e-5
GROUP_ELEMS = float((C // G) * N)  # 16384

R = mybir.dt.float32r
F32 = mybir.dt.float32
BF16 = mybir.dt.bfloat16
AF = mybir.ActivationFunctionType
ALU = mybir.AluOpType

_NC_CACHE = None


def _build():
    nc = bacc.Bacc("TRN2", target_bir_lowering=False, debug=False, num_devices=NCORES)

    q_in = nc.dram_tensor("q_in", [C, NC], R, kind="ExternalInput")
    k_in = nc.dram_tensor("k_in", [C, N], R, kind="ExternalInput")
    v_in = nc.dram_tensor("v_in", [C, N], BF16, kind="ExternalInput")
    vres_in = nc.dram_tensor("vres", [C, NC], F32, kind="ExternalInput")
    wqT_in = nc.dram_tensor("wqT", [C, C], R, kind="ExternalInput")
    wkT_in = nc.dram_tensor("wkT", [C, C], R, kind="ExternalInput")
    wvT_in = nc.dram_tensor("wvT", [C, C], BF16, kind="ExternalInput")
    woT_in = nc.dram_tensor("woT", [C, C], R, kind="ExternalInput")
    bq_in = nc.dram_tensor("bq", [C, 1], F32, kind="ExternalInput")
    bk_in = nc.dram_tensor("bk", [C, 1], F32, kind="ExternalInput")
    boe_in = nc.dram_tensor("bo_eff", [C, 1], F32, kind="ExternalInput")
    gamma_in = nc.dram_tensor("gamma", [C, 1], F32, kind="ExternalInput")
    beta_in = nc.dram_tensor("beta", [C, 1], F32, kind="ExternalInput")
    m0_in = nc.dram_tensor("m0", [G, 1], F32, kind="ExternalInput")
    m1_in = nc.dram_tensor("m1", [G, 1], F32, kind="ExternalInput")
    y_out = nc.dram_tensor("y_out", [C, NC], F32, kind="ExternalOutput")

    ones_np = np.ones((C, C), dtype=np.float32)
    e_np = np.zeros((C, G), dtype=np.float32)
    for c in range(C):
        e_np[c, c // (C // G)] = 1.0
    ones_dram = nc.inline_tensor(ones_np, name="ones128")
    e_dram = nc.inline_tensor(e_np, name="egrp")
    et_dram = nc.inline_tensor(np.ascontiguousarray(e_np.T), name="egrpT")

    with tile.TileContext(nc) as tc:
        with (
            tc.tile_pool(name="const", bufs=1) as const,
            tc.tile_pool(name="big", bufs=1) as big,
            tc.tile_pool(name="expp", bufs=3) as expp,
            tc.tile_pool(name="psum", bufs=2, space="PSUM") as psum,
            tc.tile_pool(name="ps_h", bufs=1, space="PSUM") as ps_h,
            tc.tile_pool(name="ps_db", bufs=1, space="PSUM") as ps_db,
            tc.tile_pool(name="dram", bufs=2, space="DRAM") as dram,
        ):
            # ---- constants / weights (q path first so PE can start early) ----
            wqT = const.tile([C, C], R)
            wkT = const.tile([C, C], R)
            wvT = const.tile([C, C], BF16)
            woT = const.tile([C, C], R)
            ones_sb = const.tile([C, C], R)
            e_sb = const.tile([C, G], F32)
            et_sb = const.tile([G, C], F32)
            bq_sb = const.tile([C, 1], F32)
            bk_sb = const.tile([C, 1], F32)
            boe_sb = const.tile([C, 1], F32)
            gamma_sb = const.tile([C, 1], F32)
            beta_sb = const.tile([C, 1], F32)
            m0_sb = const.tile([G, 1], F32)
            m1_sb = const.tile([G, 1], F32)
            eps_sb = const.tile([G, 1], F32)

            nc.sync.dma_start(wqT[:], wqT_in[:])
            nc.sync.dma_start(wkT[:], wkT_in[:])
            nc.sync.dma_start(wvT[:], wvT_in[:])
            q_raw = big.tile([C, NC], R)
            nc.sync.dma_start(q_raw[:], q_in[:])
            nc.sync.dma_start(bq_sb[:], bq_in[:])
            nc.sync.dma_start(bk_sb[:], bk_in[:])
            nc.sync.dma_start(ones_sb[:], ones_dram[:].bitcast(R))

            # ---- q projection: q_sb = wq @ query_chunk + bq ----
            q_sb = big.tile([C, NC], R)
            for h in range(NC // 512):
                sl = slice(h * 512, (h + 1) * 512)
                qp = psum.tile([C, 512], F32, tag="st")
                nc.tensor.matmul(qp[:], wqT[:], q_raw[:, sl], start=True, stop=True)
                nc.vector.tensor_scalar(
                    out=q_sb[:, sl], in0=qp[:],
                    scalar1=bq_sb[:], scalar2=None, op0=ALU.add,
                )

            # ---- k projection + v^T, interleaved per 512-chunk DMA ----
            k_raw = big.tile([C, N], R)
            k_sb = big.tile([C, N], R)
            v_raw = big.tile([C, N], BF16)
            v_raw3 = v_raw[:].rearrange("c (t j) -> c t j", j=128)
            vt_sb = big.tile([128, JT, C], R)
            for h in range(N // 512):
                sl = slice(h * 512, (h + 1) * 512)
                nc.sync.dma_start(k_raw[:, sl], k_in[:, sl])
                nc.sync.dma_start(v_raw[:, sl], v_in[:, sl])
                kp = psum.tile([C, 512], F32, tag="st")
                nc.tensor.matmul(kp[:], wkT[:], k_raw[:, sl], start=True, stop=True)
                nc.vector.tensor_scalar(
                    out=k_sb[:, sl], in0=kp[:],
                    scalar1=bk_sb[:], scalar2=None, op0=ALU.add,
                )
                for t in range(4 * h, 4 * h + 4):
                    vp = psum.tile([128, C], F32, tag="st")
                    nc.tensor.matmul(
                        vp[:], v_raw3[:, t, :], wvT[:], start=True, stop=True
                    )
                    nc.vector.tensor_copy(vt_sb[:, t, :], vp[:])

            # remaining small/late inputs
            nc.sync.dma_start(woT[:], woT_in[:])
            nc.sync.dma_start(e_sb[:], e_dram[:])
            nc.sync.dma_start(et_sb[:], et_dram[:])
            nc.sync.dma_start(boe_sb[:], boe_in[:])
            nc.sync.dma_start(gamma_sb[:], gamma_in[:])
            nc.sync.dma_start(beta_sb[:], beta_in[:])
            nc.sync.dma_start(m0_sb[:], m0_in[:])
            nc.sync.dma_start(m1_sb[:], m1_in[:])
            nc.vector.memset(eps_sb[:], EPS)
            vres_sb = big.tile([C, NC], F32)
            nc.sync.dma_start(vres_sb[:], vres_in[:])
            r_sb = big.tile([C, NC], F32)
            nc.vector.tensor_scalar(
                out=r_sb[:], in0=vres_sb[:],
                scalar1=boe_sb[:], scalar2=None, op0=ALU.add,
            )

            # ---- main attention loop over 32 key tiles ----
            # per tile: S^T = k_tile^T @ q (psum) -> exp (ACT, ->sbuf fp32r)
            #           h  += v^T_tile @ exp     (PSUM accumulate)
            #           db += ones    @ exp      (PSUM accumulate = denominator)
            k_sb3 = k_sb[:].rearrange("c (t j) -> c t j", j=128)
            h_ps = ps_h.tile([C, NC], F32)
            db_ps = ps_db.tile([C, NC], F32)
            for t in range(JT):
                st_ps = psum.tile([128, NC], F32, tag="st")
                for h in range(NC // 512):
                    sl = slice(h * 512, (h + 1) * 512)
                    nc.tensor.matmul(
                        st_ps[:, sl], k_sb3[:, t, :], q_sb[:, sl],
                        start=True, stop=True,
                    )
                exp_t = expp.tile([128, NC], R, tag="exp")
                nc.scalar.activation(out=exp_t[:], in_=st_ps[:], func=AF.Exp)
                for h in range(NC // 512):
                    sl = slice(h * 512, (h + 1) * 512)
                    nc.tensor.matmul(
                        h_ps[:, sl], vt_sb[:, t, :], exp_t[:, sl],
                        start=(t == 0), stop=(t == JT - 1), skip_group_check=True,
                    )
                    nc.tensor.matmul(
                        db_ps[:, sl], ones_sb[:], exp_t[:, sl],
                        start=(t == 0), stop=(t == JT - 1), skip_group_check=True,
                    )

            # ---- 1/denominator ----
            dinv_sb = big.tile([C, NC], F32)
            nc.vector.reciprocal(dinv_sb[:], db_ps[:])

            # ---- output projection; x = o * dinv + (vres + bo_eff) ----
            h_sb = big.tile([C, NC], R)
            nc.scalar.activation(out=h_sb[:], in_=h_ps[:], func=AF.Copy)
            o_ps = psum.tile([C, NC], F32, tag="st")
            for h in range(NC // 512):
                sl = slice(h * 512, (h + 1) * 512)
                nc.tensor.matmul(o_ps[:, sl], woT[:], h_sb[:, sl], start=True, stop=True)
            x_sb = big.tile([C, NC], F32)
            nc.vector.tensor_mul(x_sb[:], o_ps[:], dinv_sb[:])
            nc.vector.tensor_add(x_sb[:], x_sb[:], r_sb[:])

            # ---- groupnorm partial stats: per-channel sum / sum-of-squares ----
            rowstats = big.tile([C, 2], F32)
            nc.vector.reduce_sum(rowstats[:, 0:1], x_sb[:], axis=mybir.AxisListType.X)
            xsq_sb = big.tile([C, NC], F32)
            nc.scalar.activation(
                out=xsq_sb[:], in_=x_sb[:], func=AF.Square,
                accum_out=rowstats[:, 1:2],
            )
            gs_ps = psum.tile([G, 2], F32, tag="st")
            nc.tensor.matmul(gs_ps[:], e_sb[:], rowstats[:], start=True, stop=True)
            gs_sb = big.tile([G, 2], F32)
            nc.scalar.activation(out=gs_sb[:], in_=gs_ps[:], func=AF.Copy)

            # ---- single 8-core AllReduce of [32, 4] (both batches, masked) ----
            slab = big.tile([G, 4], F32)
            nc.vector.tensor_scalar(
                out=slab[:, 0:2], in0=gs_sb[:],
                scalar1=m0_sb[:], scalar2=None, op0=ALU.mult,
            )
            nc.vector.tensor_scalar(
                out=slab[:, 2:4], in0=gs_sb[:],
                scalar1=m1_sb[:], scalar2=None, op0=ALU.mult,
            )
            cc_in = dram.tile([G, 4], F32)
            cc_out = dram.tile([G, 4], F32)
            nc.sync.dma_start(cc_in[:], slab[:])
            nc.gpsimd.collective_compute(
                "AllReduce",
                ALU.add,
                replica_groups=[list(range(NCORES))],
                ins=[cc_in.opt()],
                outs=[cc_out.opt()],
            )
            ar_sb = big.tile([G, 4], F32)
            nc.sync.dma_start(ar_sb[:], cc_out[:])
            # own batch's [sum, sumsq] = ar[:, 0:2]*m0 + ar[:, 2:4]*m1
            own = big.tile([G, 2], F32)
            tmp2 = big.tile([G, 2], F32)
            nc.vector.tensor_scalar(
                out=own[:], in0=ar_sb[:, 0:2],
                scalar1=m0_sb[:], scalar2=None, op0=ALU.mult,
            )
            nc.vector.tensor_scalar(
                out=tmp2[:], in0=ar_sb[:, 2:4],
                scalar1=m1_sb[:], scalar2=None, op0=ALU.mult,
            )
            nc.vector.tensor_add(own[:], own[:], tmp2[:])

            # ---- group mean / rstd -> per-channel scale+bias ----
            msr = big.tile([G, 2], F32)  # [mean, rstd]
            nc.scalar.mul(msr[:, 0:1], own[:, 0:1], 1.0 / GROUP_ELEMS)
            ex2 = big.tile([G, 1], F32)
            nc.scalar.mul(ex2[:], own[:, 1:2], 1.0 / GROUP_ELEMS)
            m2 = big.tile([G, 1], F32)
            nc.vector.tensor_mul(m2[:], msr[:, 0:1], msr[:, 0:1])
            var = big.tile([G, 1], F32)
            nc.vector.tensor_sub(var[:], ex2[:], m2[:])
            sd = big.tile([G, 1], F32)
            nc.scalar.activation(
                out=sd[:], in_=var[:], func=AF.Sqrt, bias=eps_sb[:], scale=1.0
            )
            nc.vector.reciprocal(msr[:, 1:2], sd[:])
            exp_ps = psum.tile([C, 2], F32, tag="st")
            nc.tensor.matmul(exp_ps[:], et_sb[:], msr[:], start=True, stop=True)
            mr_sb = big.tile([C, 2], F32)
            nc.scalar.activation(out=mr_sb[:], in_=exp_ps[:], func=AF.Copy)
            fs_sb = big.tile([C, 1], F32)
            nc.vector.tensor_mul(fs_sb[:], mr_sb[:, 1:2], gamma_sb[:])
            fb_sb = big.tile([C, 1], F32)
            nc.vector.tensor_mul(fb_sb[:], mr_sb[:, 0:1], fs_sb[:])
            nc.vector.tensor_sub(fb_sb[:], beta_sb[:], fb_sb[:])

            # ---- out = silu(fs * x + fb) ----
            y_sb = big.tile([C, NC], F32)
            nc.scalar.activation(
                out=y_sb[:], in_=x_sb[:], func=AF.Silu, bias=fb_sb[:], scale=fs_sb[:]
            )
            nc.sync.dma_start(y_out[:], y_sb[:])

    nc.compile()
    return nc


def _get_nc():
    global _NC_CACHE
    if _NC_CACHE is None:
        _NC_CACHE = _build()
    return _NC_CACHE


def _in_maps(query, key, value, wq, bq, wk, bk, wv, bv, wo, bo, gamma, beta):
    f32 = lambda a: np.ascontiguousarray(np.asarray(a, dtype=np.float32))
    q = f32(query).reshape(B, C, N)
    k = f32(key).reshape(B, C, N)
    v = f32(value).reshape(B, C, N)
    wq, wk, wv, wo = f32(wq), f32(wk), f32(wv), f32(wo)
    bo_eff = (wo @ f32(bv).reshape(C) + f32(bo).reshape(C)).astype(np.float32)

    shared = {
        "wqT": np.ascontiguousarray(wq.T),
        "wkT": np.ascontiguousarray(wk.T),
        "wvT": np.ascontiguousarray(wv.T).astype(ml_dtypes.bfloat16),
        "woT": np.ascontiguousarray(wo.T),
        "bq": f32(bq).reshape(C, 1),
        "bk": f32(bk).reshape(C, 1),
        "bo_eff": bo_eff.reshape(C, 1),
        "gamma": f32(gamma).reshape(C, 1),
        "beta": f32(beta).reshape(C, 1),
    }
    maps = []
    for p in range(NCORES):
        b, ch = divmod(p, CHUNKS)
        sl = slice(ch * NC, (ch + 1) * NC)
        m0 = np.full((G, 1), 1.0 if b == 0 else 0.0, np.float32)
        m1 = np.full((G, 1), 1.0 if b == 1 else 0.0, np.float32)
        maps.append(
            {
                "q_in": np.ascontiguousarray(q[b][:, sl]),
                "k_in": k[b],
                "v_in": v[b].astype(ml_dtypes.bfloat16),
                "vres": np.ascontiguousarray(v[b][:, sl]),
                "m0": m0,
                "m1": m1,
                **shared,
            }
        )
    return maps


def kernel(query, key, value, wq, bq, wk, bk, wv, bv, wo, bo, gamma, beta):
    nc = _get_nc()
    maps = _in_maps(query, key, value, wq, bq, wk, bk, wv, bv, wo, bo, gamma, beta)
    res = run_bass_kernel_spmd(nc, maps, list(range(NCORES)))
    out = np.empty((B, C, N), dtype=np.float32)
    for p in range(NCORES):
        b, ch = divmod(p, CHUNKS)
        out[b][:, ch * NC : (ch + 1) * NC] = res.results[p]["y_out"]
    return out.reshape(B, C, 16, 16, 16)
